# revision 24
# baseline (speedup 1.0000x reference)
"""Causal single-head attention (projections + softmax(QK^T)V) on 8 TRN2 cores.

Sharding: pure data parallelism over the batch dim (B=8 -> one batch element
per NeuronCore). Each core runs an identical Bass/Tile program on its shard.

Mixed-precision dataflow (chosen from a numpy error study against the fp32
reference; gate is rel_err < 2e-2):
  - query/key inputs stream as fp8 e4m3 [E,S] (halves their HBM traffic) and
    their projections run as fp8 DoubleRow matmuls (two 128-deep e-tiles
    contracted per pass -> 2x PE throughput).  Wq/Wk are pre-scaled by 32 on
    the host before e4m3 quantization so the uniform(-1/32,1/32) weights land
    in e4m3's normal range; the resulting 32*32=1024 score scale folds into
    the exp() scale constant for free.  q/k land in PSUM fp32 and evict to
    fp16, so the score matmul itself stays fp16 (no re-quantization).
  - the value path is the precision-critical one (v errors pass straight to
    the output for near-diagonal queries), so value streams fp16 and the
    v-projection + AV matmuls stay fp16 end to end.
  Measured (numpy sim, bit-faithful on the fp16 baseline): rel_err ~1.67e-2.

Per-core dataflow (everything transposed so no on-chip transposes of the big
score matrix are needed):
  - qT/kT/vT [d=128, S] = W_xT.T @ xT accumulated in PSUM fp32, bias added
    on the VectorE eviction to SBUF fp16,
  - scores^T block [k,q] = kT_blk.T @ qT, exp on ScalarE with fused
    1/(1024*sqrt(d)) scale (no max-subtraction: |scores_true/sqrt(d)| <= ~4
    so exp cannot overflow),
  - causal mask applied only on diagonal 128x128 blocks (entries with q < k
    in lower tiles are never read by the AV stage),
  - out[q, dv] and the softmax denominator come from one PE accumulation:
    [num | den] = exp_blk.T @ [v | 1]; normalize on VectorE; DMA out fp16,
    upcast to fp32 on the host.

The computation is pipelined in NJ rounds over 256-wide s-chunks: each round
loads its q/k/v input chunks (the sync HWDGE ring carries only these, in
order, so chunk DMAs complete just-in-time at full HBM bandwidth), projects
them, and immediately runs the newly-enabled score/exp/AV work.
"""

import math

import numpy as np

import concourse.bass as bass  # noqa: F401  (registers AP machinery)
import concourse.tile as tile
from concourse import bacc, mybir
from concourse.bass_utils import run_bass_kernel_spmd

B, S, E = 8, 2048, 1024
DQ, DV = 128, 128
P = 128
EO = E // P          # 8 e-chunks
ST = S // P          # 16 sequence tiles of 128
NCH = 256            # s-chunk width per pipeline round
NJ = S // NCH        # 8 s-chunks
TPR = ST // NJ       # sequence tiles per round (2)
NCORES = 8
WS = 32.0            # host-side Wq/Wk scale before fp8 quantization
SCALE = 1.0 / math.sqrt(DQ) / (WS * WS)

f8 = mybir.dt.float8e4
f16 = mybir.dt.float16
f32 = mybir.dt.float32
DR = mybir.MatmulPerfMode.DoubleRow

_CACHE = {}
LAST_RESULT = None  # BassKernelResults of the most recent run (for profiling)


def _build_nc():
    nc = bacc.Bacc("TRN2", target_bir_lowering=False, debug=False)

    qkx_e = nc.declare_dram_parameter("qkx", [P, NJ, 2, EO, NCH], f8, isOutput=False)
    vx_e = nc.declare_dram_parameter("vx", [P, NJ, EO, NCH], f16, isOutput=False)
    wqk_e = nc.declare_dram_parameter("wqk", [P, 2, EO, DQ], f8, isOutput=False)
    wv_e = nc.declare_dram_parameter("wv", [P, EO, DV], f16, isOutput=False)
    bias3_e = nc.declare_dram_parameter("bias3", [P, 3], f32, isOutput=False)
    idmask_e = nc.declare_dram_parameter("idmask", [P, 2, P], f16, isOutput=False)
    out_e = nc.declare_dram_parameter("out", [P, ST, DV], f16, isOutput=True)

    Exp = mybir.ActivationFunctionType.Exp

    with (
        tile.TileContext(nc) as tc,
        tc.tile_pool(name="consts", bufs=1) as consts,
        tc.tile_pool(name="inx", bufs=9) as inx,
        tc.tile_pool(name="acts", bufs=1) as acts,
        tc.tile_pool(name="outp", bufs=10) as outp,
        tc.tile_pool(name="pp", bufs=3, space="PSUM") as pp,
        tc.tile_pool(name="ps_s", bufs=3, space="PSUM") as ps_s_pool,
        tc.tile_pool(name="ps_n", bufs=2, space="PSUM") as ps_n_pool,
    ):
        # Consts ride the scalar HWDGE / gpsimd SWDGE so the sync HWDGE ring
        # carries nothing but streamed input chunks (no head-of-line waits).
        # wq leads: it gates the very first projection matmul; ident/mask are
        # not needed until mid-round-0 so they trail the weights.
        wqk_sb = consts.tile([P, 2, EO, DQ], f8, tag="wqk")
        nc.scalar.dma_start(wqk_sb[:], wqk_e.ap())
        wv_sb = consts.tile([P, EO, DQ], f16, tag="wv")
        nc.scalar.dma_start(wv_sb[:], wv_e.ap())
        bias3 = consts.tile([P, 3], f32, tag="bias3")
        nc.gpsimd.dma_start(bias3[:], bias3_e.ap())
        idmask = consts.tile([P, 2, P], f16, tag="idmask")
        nc.gpsimd.dma_start(idmask[:], idmask_e.ap())
        id_sb = idmask[:, 0, :]
        mask_sb = idmask[:, 1, :]
        b_sb = {
            "bq": bias3[:, 0:1],
            "bk": bias3[:, 1:2],
            "bv": bias3[:, 2:3],
        }

        # PE warm-up: the first real matmul cannot start before the first
        # input chunk lands (~3.5us), and a cold PE runs slow for its first
        # ~3us of activity.  A DMA-independent burst of matmuls on a memset
        # operand spans the wait so the HAM clock gate is warm when the
        # projections begin.  One junk DVE copy consumes the result so DCE
        # keeps it.
        wu_in = consts.tile([P, NCH], f16, tag="wu_in")
        nc.vector.memset(wu_in[:], 1.0)
        wu_ps = pp.tile([P, NCH], f32, tag="pp")
        junk = consts.tile([P, P], f32, tag="junk")
        for _ in range(12):
            nc.tensor.matmul(wu_ps[:], wu_in[:, :P], wu_in[:], start=True, stop=True)

        qT = acts.tile([P, S], f16, tag="qT")
        kT = acts.tile([P, S], f16, tag="kT")
        vT = acts.tile([P, S], f16, tag="vT")
        v_ext = acts.tile([P, ST, DV + 1], f16, tag="vex")
        nc.vector.memset(v_ext[:, :, DV : DV + 1], 1.0)
        E_big = acts.tile([P, ST, S], f16, tag="exp")
        obuf = acts.tile([P, ST, DV], f16, tag="obuf")

        # ring plan (per 128-desc transfer the ring is busy ~1.3us, so rings
        # are the scarce resource early):
        #   sync:   packed qk chunks (one transfer carries q_j AND k_j), stores
        #   scalar: wqk, wv, then v chunks
        #   gpsimd: packed bias3 + packed ident/mask
        def proj_qk(j):
            xc = inx.tile([P, 2, EO, NCH], f8, tag="qk")
            nc.sync.dma_start(xc[:], qkx_e.ap()[:, j])
            sl = slice(j * NCH, (j + 1) * NCH)
            for half, bt, dst in ((0, b_sb["bq"], qT), (1, b_sb["bk"], kT)):
                ps = pp.tile([P, NCH], f32, tag="pp")
                # fp8 DoubleRow: two 128-deep e-tiles per pass
                for h in range(EO // 2):
                    nc.tensor.matmul(
                        ps[:],
                        wqk_sb[:, half, 2 * h : 2 * h + 2, :],
                        xc[:, half, 2 * h : 2 * h + 2, :],
                        start=(h == 0),
                        stop=(h == EO // 2 - 1),
                        perf_mode=DR,
                    )
                nc.vector.tensor_scalar_add(dst[:, sl], ps[:], bt)

        def proj_v(j):
            xc = inx.tile([P, EO, NCH], f16, tag="inx2")
            nc.scalar.dma_start(xc[:], vx_e.ap()[:, j])
            ps = pp.tile([P, NCH], f32, tag="pp")
            sl = slice(j * NCH, (j + 1) * NCH)
            for eo in range(EO):
                nc.tensor.matmul(
                    ps[:],
                    wv_sb[:, eo, :],
                    xc[:, eo, :],
                    start=(eo == 0),
                    stop=(eo == EO - 1),
                )
            nc.vector.tensor_scalar_add(vT[:, sl], ps[:], b_sb["bv"])

        def v_round(r):
            # v chunk + projection + [s, dv] v blocks + diagonal masks for
            # round r (the v DMA stays in its own round so the stream order
            # is unchanged)
            proj_v(r)
            for st in range(TPR * r, TPR * (r + 1)):
                tp = ps_n_pool.tile([P, P], f16, tag="ps_n")
                nc.tensor.transpose(tp[:], vT[:, st * P : (st + 1) * P], id_sb)
                nc.vector.tensor_copy(v_ext[:, st, 0:DV], tp[:])
            for kt in range(TPR * r, TPR * (r + 1)):
                d0 = kt * P
                nc.vector.tensor_mul(
                    E_big[:, kt, d0 : d0 + P],
                    E_big[:, kt, d0 : d0 + P],
                    mask_sb,
                )

        def av_round(r):
            # AV + normalize for round r's q tiles; runs one round behind its
            # scores (at the TOP of round r+1) so the ScalarE exp stream is
            # never on the AV critical path and the in-order PE has ready
            # work to chew while round r+1's chunks are still streaming in
            for qt in range(TPR * r, TPR * (r + 1)):
                pn = ps_n_pool.tile([P, DV + 1], f32, tag="ps_n")
                for kt in range(qt + 1):
                    nc.tensor.matmul(
                        pn[:],
                        E_big[:, kt, qt * P : (qt + 1) * P],
                        v_ext[:, kt, :],
                        start=(kt == 0),
                        stop=(kt == qt),
                    )
                rec = outp.tile([P, 1], f32, tag="rec")
                nc.vector.reciprocal(rec[:], pn[:, DV : DV + 1])
                nc.vector.tensor_scalar_mul(obuf[:, qt, :], pn[:, 0:DV], rec[:])

        for j in range(NJ):
            with nc.named_scope(f"round{j}"):
                if j >= 1:
                    av_round(j - 1)
                # qk projections first: scores depend on them, and their
                # packed chunk gets the early DMA bandwidth; v follows.
                proj_qk(j)

                # rounds 0-1 are DMA-paced: filler matmuls keep the PE busy
                # enough that the HAM activity monitor never re-throttles the
                # clock while the input stream catches up
                if j <= 1:
                    for _ in range(6 if j == 0 else 4):
                        nc.tensor.matmul(
                            wu_ps[:], wu_in[:, :P], wu_in[:], start=True, stop=True
                        )
                    if j == 1:
                        nc.vector.tensor_copy(junk[:], wu_ps[:, :P])

                # scores^T for q-chunk j against all causal k tiles; two
                # k-tiles share one PSUM pair-tile so a single exp call covers
                # both (amortizes ACT per-instruction overhead).
                sl = slice(j * NCH, (j + 1) * NCH)
                for kt in range(0, TPR * (j + 1), 2):
                    ps = ps_s_pool.tile([P, 2, NCH], f32, tag="ps_s")
                    for u in range(2):
                        nc.tensor.matmul(
                            ps[:, u, :],
                            kT[:, (kt + u) * P : (kt + u + 1) * P],
                            qT[:, sl],
                            start=True,
                            stop=True,
                        )
                    nc.scalar.activation(
                        E_big[:, kt : kt + 2, sl], ps[:], Exp, scale=SCALE
                    )

                v_round(j)
                if j == NJ - 1:
                    # tiles 0-13 are normalized by now; the sync ring is idle
                    nc.sync.dma_start(out_e.ap()[:, : ST - 2], obuf[:, : ST - 2])

        with nc.named_scope("avtail"):
            av_round(NJ - 1)
            nc.sync.dma_start(out_e.ap()[:, ST - 2 :], obuf[:, ST - 2 :])

    nc.compile()
    return nc


def _get_nc():
    if "nc" not in _CACHE:
        _CACHE["nc"] = _build_nc()
    return _CACHE["nc"]


NP_F8 = mybir.dt.np(f8)


def _prep_consts(Wq, bq, Wk, bk, Wv, bv):
    def prep_w(W, scale, npdt):  # [D, E] f32 -> (scale*W).T [E, D] -> [ei, eo, D]
        WT = (scale * W).T.astype(npdt)  # [E, D]
        return np.ascontiguousarray(WT.reshape(EO, P, -1).transpose(1, 0, 2))

    consts = {
        "wqk": np.ascontiguousarray(
            np.stack([prep_w(Wq, WS, NP_F8), prep_w(Wk, WS, NP_F8)], axis=1)
        ),
        "wv": prep_w(Wv, 1.0, np.float16),
        "bias3": np.ascontiguousarray(
            np.stack([WS * bq, WS * bk, bv], axis=1).astype(np.float32)
        ),
        "idmask": np.ascontiguousarray(
            np.stack(
                [np.eye(P, dtype=np.float16), np.triu(np.ones((P, P), np.float16))],
                axis=1,
            )
        ),
    }
    return consts


def _prep_x(x, npdt):  # [S, E] f32 -> xT [E, S] -> [ei, j, eo, s_in_chunk]
    xT = x.astype(npdt).T  # [E, S]
    x4 = xT.reshape(EO, P, NJ, NCH)  # [eo, ei, j, s]
    return np.ascontiguousarray(x4.transpose(1, 2, 0, 3))


def kernel(query, key_in, value, Wq, bq, Wk, bk, Wv, bv):
    global LAST_RESULT
    query = np.asarray(query, dtype=np.float32)
    key_in = np.asarray(key_in, dtype=np.float32)
    value = np.asarray(value, dtype=np.float32)
    consts = _prep_consts(
        np.asarray(Wq), np.asarray(bq), np.asarray(Wk),
        np.asarray(bk), np.asarray(Wv), np.asarray(bv),
    )
    in_maps = []
    for b in range(NCORES):
        m = dict(consts)
        m["qkx"] = np.ascontiguousarray(
            np.stack(
                [_prep_x(query[b], NP_F8), _prep_x(key_in[b], NP_F8)], axis=2
            )
        )
        m["vx"] = _prep_x(value[b], np.float16)
        in_maps.append(m)

    nc = _get_nc()
    res = run_bass_kernel_spmd(nc, in_maps, core_ids=list(range(NCORES)))
    LAST_RESULT = res
    outs = []
    for i in range(NCORES):
        o = res.results[i]["out"]  # [P, ST, DV] with s = st*P + ei
        outs.append(o.transpose(1, 0, 2).reshape(S, DV))
    return np.stack(outs, axis=0).astype(np.float32)


# revision 25
# speedup vs baseline: 1.0211x; 1.0211x over previous
"""Causal single-head attention (projections + softmax(QK^T)V) on 8 TRN2 cores.

Sharding: pure data parallelism over the batch dim (B=8 -> one batch element
per NeuronCore). Each core runs an identical Bass/Tile program on its shard.

Mixed-precision dataflow (chosen from a numpy error study against the fp32
reference; gate is rel_err < 2e-2):
  - query/key inputs stream as fp8 e4m3 [E,S] (halves their HBM traffic) and
    their projections run as fp8 DoubleRow matmuls (two 128-deep e-tiles
    contracted per pass -> 2x PE throughput).  Wq/Wk are pre-scaled by 32 on
    the host before e4m3 quantization so the uniform(-1/32,1/32) weights land
    in e4m3's normal range; the resulting 32*32=1024 score scale folds into
    the exp() scale constant for free.  q/k land in PSUM fp32 and evict to
    fp16, so the score matmul itself stays fp16 (no re-quantization).
  - the value path is the precision-critical one (v errors pass straight to
    the output for near-diagonal queries), so value streams fp16 and the
    v-projection + AV matmuls stay fp16 end to end.
  Measured (numpy sim, bit-faithful on the fp16 baseline): rel_err ~1.67e-2.

Per-core dataflow (everything transposed so no on-chip transposes of the big
score matrix are needed):
  - qT/kT/vT [d=128, S] = W_xT.T @ xT accumulated in PSUM fp32, bias added
    on the VectorE eviction to SBUF fp16,
  - scores^T block [k,q] = kT_blk.T @ qT, exp on ScalarE with fused
    1/(1024*sqrt(d)) scale (no max-subtraction: |scores_true/sqrt(d)| <= ~4
    so exp cannot overflow),
  - causal mask applied only on diagonal 128x128 blocks (entries with q < k
    in lower tiles are never read by the AV stage),
  - out[q, dv] and the softmax denominator come from one PE accumulation:
    [num | den] = exp_blk.T @ [v | 1]; normalize on VectorE; DMA out fp16,
    upcast to fp32 on the host.

The computation is pipelined in NJ rounds over 256-wide s-chunks: each round
loads its q/k/v input chunks (the sync HWDGE ring carries only these, in
order, so chunk DMAs complete just-in-time at full HBM bandwidth), projects
them, and immediately runs the newly-enabled score/exp/AV work.
"""

import math

import numpy as np

import concourse.bass as bass  # noqa: F401  (registers AP machinery)
import concourse.tile as tile
from concourse import bacc, mybir
from concourse.bass_utils import run_bass_kernel_spmd

B, S, E = 8, 2048, 1024
DQ, DV = 128, 128
P = 128
EO = E // P          # 8 e-chunks
ST = S // P          # 16 sequence tiles of 128
NCH = 256            # s-chunk width per pipeline round
NJ = S // NCH        # 8 s-chunks
TPR = ST // NJ       # sequence tiles per round (2)
NCORES = 8
WS = 32.0            # host-side Wq/Wk scale before fp8 quantization
SCALE = 1.0 / math.sqrt(DQ) / (WS * WS)

f8 = mybir.dt.float8e4
f16 = mybir.dt.float16
f32 = mybir.dt.float32
DR = mybir.MatmulPerfMode.DoubleRow

_CACHE = {}
LAST_RESULT = None  # BassKernelResults of the most recent run (for profiling)


def _build_nc():
    nc = bacc.Bacc("TRN2", target_bir_lowering=False, debug=False)

    qkx_e = nc.declare_dram_parameter("qkx", [P, NJ, 2, EO, NCH], f8, isOutput=False)
    vx_e = nc.declare_dram_parameter("vx", [P, NJ, EO, NCH], f16, isOutput=False)
    wqk_e = nc.declare_dram_parameter("wqk", [P, 2, EO, DQ], f8, isOutput=False)
    wv_e = nc.declare_dram_parameter("wv", [P, EO, DV], f16, isOutput=False)
    bias3_e = nc.declare_dram_parameter("bias3", [P, 3], f32, isOutput=False)
    idmask_e = nc.declare_dram_parameter("idmask", [P, 2, P], f16, isOutput=False)
    out_e = nc.declare_dram_parameter("out", [P, ST, DV], f16, isOutput=True)

    Exp = mybir.ActivationFunctionType.Exp

    with (
        tile.TileContext(nc) as tc,
        tc.tile_pool(name="consts", bufs=1) as consts,
        tc.tile_pool(name="inx", bufs=9) as inx,
        tc.tile_pool(name="acts", bufs=1) as acts,
        tc.tile_pool(name="outp", bufs=10) as outp,
        tc.tile_pool(name="pp", bufs=3, space="PSUM") as pp,
        tc.tile_pool(name="ps_s", bufs=3, space="PSUM") as ps_s_pool,
        tc.tile_pool(name="ps_n", bufs=2, space="PSUM") as ps_n_pool,
    ):
        # Consts ride the scalar HWDGE / gpsimd SWDGE so the sync HWDGE ring
        # carries nothing but streamed input chunks (no head-of-line waits).
        # wq leads: it gates the very first projection matmul; ident/mask are
        # not needed until mid-round-0 so they trail the weights.
        wqk_sb = consts.tile([P, 2, EO, DQ], f8, tag="wqk")
        nc.scalar.dma_start(wqk_sb[:], wqk_e.ap())
        wv_sb = consts.tile([P, EO, DQ], f16, tag="wv")
        nc.scalar.dma_start(wv_sb[:], wv_e.ap())
        bias3 = consts.tile([P, 3], f32, tag="bias3")
        nc.gpsimd.dma_start(bias3[:], bias3_e.ap())
        idmask = consts.tile([P, 2, P], f16, tag="idmask")
        nc.gpsimd.dma_start(idmask[:], idmask_e.ap())
        id_sb = idmask[:, 0, :]
        mask_sb = idmask[:, 1, :]
        b_sb = {
            "bq": bias3[:, 0:1],
            "bk": bias3[:, 1:2],
            "bv": bias3[:, 2:3],
        }

        # PE warm-up: the first real matmul cannot start before the first
        # input chunk lands (~3.5us), and a cold PE runs slow for its first
        # ~3us of activity.  A DMA-independent burst of matmuls on a memset
        # operand spans the wait so the HAM clock gate is warm when the
        # projections begin.  One junk DVE copy consumes the result so DCE
        # keeps it.
        wu_in = consts.tile([P, NCH], f16, tag="wu_in")
        nc.vector.memset(wu_in[:], 1.0)
        wu_ps = pp.tile([P, NCH], f32, tag="pp")
        junk = consts.tile([P, P], f32, tag="junk")
        for _ in range(12):
            nc.tensor.matmul(wu_ps[:], wu_in[:, :P], wu_in[:], start=True, stop=True)

        qT = acts.tile([P, S], f16, tag="qT")
        kT = acts.tile([P, S], f16, tag="kT")
        vT = acts.tile([P, S], f16, tag="vT")
        v_ext = acts.tile([P, ST, DV + 1], f16, tag="vex")
        nc.vector.memset(v_ext[:, :, DV : DV + 1], 1.0)
        E_big = acts.tile([P, ST, S], f16, tag="exp")
        obuf = acts.tile([P, ST, DV], f16, tag="obuf")

        # ring plan (per 128-desc transfer the ring is busy ~1.3us, so rings
        # are the scarce resource early):
        #   sync:   packed qk chunks (one transfer carries q_j AND k_j), stores
        #   scalar: wqk, wv, then v chunks
        #   gpsimd: packed bias3 + packed ident/mask
        def proj_qk(j):
            xc = inx.tile([P, 2, EO, NCH], f8, tag="qk")
            nc.sync.dma_start(xc[:], qkx_e.ap()[:, j])
            sl = slice(j * NCH, (j + 1) * NCH)
            for half, bt, dst in ((0, b_sb["bq"], qT), (1, b_sb["bk"], kT)):
                ps = pp.tile([P, NCH], f32, tag="pp")
                # fp8 DoubleRow: two 128-deep e-tiles per pass
                for h in range(EO // 2):
                    nc.tensor.matmul(
                        ps[:],
                        wqk_sb[:, half, 2 * h : 2 * h + 2, :],
                        xc[:, half, 2 * h : 2 * h + 2, :],
                        start=(h == 0),
                        stop=(h == EO // 2 - 1),
                        perf_mode=DR,
                    )
                nc.vector.tensor_scalar_add(dst[:, sl], ps[:], bt)

        vtiles = {}

        def v_dma(j):
            # the trigger instruction runs on the Scalar engine's queue, so
            # it must be issued BEFORE that round's exps (578ns each) or the
            # v stream falls behind the PE
            xc = inx.tile([P, EO, NCH], f16, tag="inx2")
            nc.scalar.dma_start(xc[:], vx_e.ap()[:, j])
            vtiles[j] = xc

        def proj_v(j):
            xc = vtiles.pop(j)
            ps = pp.tile([P, NCH], f32, tag="pp")
            sl = slice(j * NCH, (j + 1) * NCH)
            for eo in range(EO):
                nc.tensor.matmul(
                    ps[:],
                    wv_sb[:, eo, :],
                    xc[:, eo, :],
                    start=(eo == 0),
                    stop=(eo == EO - 1),
                )
            nc.vector.tensor_scalar_add(vT[:, sl], ps[:], b_sb["bv"])

        def v_round(r):
            # v chunk + projection + [s, dv] v blocks + diagonal masks for
            # round r (the v DMA stays in its own round so the stream order
            # is unchanged)
            proj_v(r)
            for st in range(TPR * r, TPR * (r + 1)):
                tp = ps_n_pool.tile([P, P], f16, tag="ps_n")
                nc.tensor.transpose(tp[:], vT[:, st * P : (st + 1) * P], id_sb)
                nc.vector.tensor_copy(v_ext[:, st, 0:DV], tp[:])
            for kt in range(TPR * r, TPR * (r + 1)):
                d0 = kt * P
                nc.vector.tensor_mul(
                    E_big[:, kt, d0 : d0 + P],
                    E_big[:, kt, d0 : d0 + P],
                    mask_sb,
                )

        def av_round(r):
            # AV + normalize for round r's q tiles; runs one round behind its
            # scores (at the TOP of round r+1) so the ScalarE exp stream is
            # never on the AV critical path and the in-order PE has ready
            # work to chew while round r+1's chunks are still streaming in
            for qt in range(TPR * r, TPR * (r + 1)):
                pn = ps_n_pool.tile([P, DV + 1], f32, tag="ps_n")
                for kt in range(qt + 1):
                    nc.tensor.matmul(
                        pn[:],
                        E_big[:, kt, qt * P : (qt + 1) * P],
                        v_ext[:, kt, :],
                        start=(kt == 0),
                        stop=(kt == qt),
                    )
                rec = outp.tile([P, 1], f32, tag="rec")
                nc.vector.reciprocal(rec[:], pn[:, DV : DV + 1])
                nc.vector.tensor_scalar_mul(obuf[:, qt, :], pn[:, 0:DV], rec[:])

        for j in range(NJ):
            with nc.named_scope(f"round{j}"):
                if j == 0:
                    v_dma(0)
                if j < NJ - 1:
                    v_dma(j + 1)
                if j >= 1:
                    av_round(j - 1)
                # qk projections first: scores depend on them, and their
                # packed chunk gets the early DMA bandwidth; v follows.
                proj_qk(j)

                # rounds 0-1 are DMA-paced: filler matmuls keep the PE busy
                # enough that the HAM activity monitor never re-throttles the
                # clock while the input stream catches up
                if j <= 1:
                    for _ in range(6 if j == 0 else 4):
                        nc.tensor.matmul(
                            wu_ps[:], wu_in[:, :P], wu_in[:], start=True, stop=True
                        )
                    if j == 1:
                        nc.vector.tensor_copy(junk[:], wu_ps[:, :P])

                # scores^T for q-chunk j against all causal k tiles; two
                # k-tiles share one PSUM pair-tile so a single exp call covers
                # both (amortizes ACT per-instruction overhead).
                sl = slice(j * NCH, (j + 1) * NCH)
                for kt in range(0, TPR * (j + 1), 2):
                    ps = ps_s_pool.tile([P, 2, NCH], f32, tag="ps_s")
                    for u in range(2):
                        nc.tensor.matmul(
                            ps[:, u, :],
                            kT[:, (kt + u) * P : (kt + u + 1) * P],
                            qT[:, sl],
                            start=True,
                            stop=True,
                        )
                    nc.scalar.activation(
                        E_big[:, kt : kt + 2, sl], ps[:], Exp, scale=SCALE
                    )

                v_round(j)
                if j == NJ - 1:
                    # tiles 0-13 are normalized by now; the sync ring is idle
                    nc.sync.dma_start(out_e.ap()[:, : ST - 2], obuf[:, : ST - 2])

        with nc.named_scope("avtail"):
            av_round(NJ - 1)
            nc.sync.dma_start(out_e.ap()[:, ST - 2 :], obuf[:, ST - 2 :])

    nc.compile()
    return nc


def _get_nc():
    if "nc" not in _CACHE:
        _CACHE["nc"] = _build_nc()
    return _CACHE["nc"]


NP_F8 = mybir.dt.np(f8)


def _prep_consts(Wq, bq, Wk, bk, Wv, bv):
    def prep_w(W, scale, npdt):  # [D, E] f32 -> (scale*W).T [E, D] -> [ei, eo, D]
        WT = (scale * W).T.astype(npdt)  # [E, D]
        return np.ascontiguousarray(WT.reshape(EO, P, -1).transpose(1, 0, 2))

    consts = {
        "wqk": np.ascontiguousarray(
            np.stack([prep_w(Wq, WS, NP_F8), prep_w(Wk, WS, NP_F8)], axis=1)
        ),
        "wv": prep_w(Wv, 1.0, np.float16),
        "bias3": np.ascontiguousarray(
            np.stack([WS * bq, WS * bk, bv], axis=1).astype(np.float32)
        ),
        "idmask": np.ascontiguousarray(
            np.stack(
                [np.eye(P, dtype=np.float16), np.triu(np.ones((P, P), np.float16))],
                axis=1,
            )
        ),
    }
    return consts


def _prep_x(x, npdt):  # [S, E] f32 -> xT [E, S] -> [ei, j, eo, s_in_chunk]
    xT = x.astype(npdt).T  # [E, S]
    x4 = xT.reshape(EO, P, NJ, NCH)  # [eo, ei, j, s]
    return np.ascontiguousarray(x4.transpose(1, 2, 0, 3))


def kernel(query, key_in, value, Wq, bq, Wk, bk, Wv, bv):
    global LAST_RESULT
    query = np.asarray(query, dtype=np.float32)
    key_in = np.asarray(key_in, dtype=np.float32)
    value = np.asarray(value, dtype=np.float32)
    consts = _prep_consts(
        np.asarray(Wq), np.asarray(bq), np.asarray(Wk),
        np.asarray(bk), np.asarray(Wv), np.asarray(bv),
    )
    in_maps = []
    for b in range(NCORES):
        m = dict(consts)
        m["qkx"] = np.ascontiguousarray(
            np.stack(
                [_prep_x(query[b], NP_F8), _prep_x(key_in[b], NP_F8)], axis=2
            )
        )
        m["vx"] = _prep_x(value[b], np.float16)
        in_maps.append(m)

    nc = _get_nc()
    res = run_bass_kernel_spmd(nc, in_maps, core_ids=list(range(NCORES)))
    LAST_RESULT = res
    outs = []
    for i in range(NCORES):
        o = res.results[i]["out"]  # [P, ST, DV] with s = st*P + ei
        outs.append(o.transpose(1, 0, 2).reshape(S, DV))
    return np.stack(outs, axis=0).astype(np.float32)


# revision 26
# speedup vs baseline: 1.0560x; 1.0341x over previous
"""Causal single-head attention (projections + softmax(QK^T)V) on 8 TRN2 cores.

Sharding: pure data parallelism over the batch dim (B=8 -> one batch element
per NeuronCore). Each core runs an identical Bass/Tile program on its shard.

Mixed-precision dataflow (chosen from a numpy error study against the fp32
reference; gate is rel_err < 2e-2):
  - query/key inputs stream as fp8 e4m3 [E,S] (halves their HBM traffic) and
    their projections run as fp8 DoubleRow matmuls (two 128-deep e-tiles
    contracted per pass -> 2x PE throughput).  Wq/Wk are pre-scaled by 32 on
    the host before e4m3 quantization so the uniform(-1/32,1/32) weights land
    in e4m3's normal range; the resulting 32*32=1024 score scale folds into
    the exp() scale constant for free.  q/k land in PSUM fp32 and evict to
    fp16, so the score matmul itself stays fp16 (no re-quantization).
  - the value path is the precision-critical one (v errors pass straight to
    the output for near-diagonal queries), so value streams fp16 and the
    v-projection + AV matmuls stay fp16 end to end.
  Measured (numpy sim, bit-faithful on the fp16 baseline): rel_err ~1.67e-2.

Per-core dataflow (everything transposed so no on-chip transposes of the big
score matrix are needed):
  - qT/kT/vT [d=128, S] = W_xT.T @ xT accumulated in PSUM fp32, bias added
    on the VectorE eviction to SBUF fp16,
  - scores^T block [k,q] = kT_blk.T @ qT, exp on ScalarE with fused
    1/(1024*sqrt(d)) scale (no max-subtraction: |scores_true/sqrt(d)| <= ~4
    so exp cannot overflow),
  - causal mask applied only on diagonal 128x128 blocks (entries with q < k
    in lower tiles are never read by the AV stage),
  - out[q, dv] and the softmax denominator come from one PE accumulation:
    [num | den] = exp_blk.T @ [v | 1]; normalize on VectorE; DMA out fp16,
    upcast to fp32 on the host.

The computation is pipelined in NJ rounds over 256-wide s-chunks: each round
loads its q/k/v input chunks (the sync HWDGE ring carries only these, in
order, so chunk DMAs complete just-in-time at full HBM bandwidth), projects
them, and immediately runs the newly-enabled score/exp/AV work.
"""

import math

import numpy as np

import concourse.bass as bass  # noqa: F401  (registers AP machinery)
import concourse.tile as tile
from concourse import bacc, mybir
from concourse.bass_utils import run_bass_kernel_spmd

B, S, E = 8, 2048, 1024
DQ, DV = 128, 128
P = 128
EO = E // P          # 8 e-chunks
ST = S // P          # 16 sequence tiles of 128
NCH = 256            # s-chunk width per pipeline round
NJ = S // NCH        # 8 s-chunks
TPR = ST // NJ       # sequence tiles per round (2)
NCORES = 8
WS = 32.0            # host-side Wq/Wk scale before fp8 quantization
SCALE = 1.0 / math.sqrt(DQ) / (WS * WS)

f8 = mybir.dt.float8e4
f16 = mybir.dt.float16
f32 = mybir.dt.float32
DR = mybir.MatmulPerfMode.DoubleRow

_CACHE = {}
LAST_RESULT = None  # BassKernelResults of the most recent run (for profiling)


def _build_nc():
    nc = bacc.Bacc("TRN2", target_bir_lowering=False, debug=False)

    qkx_e = nc.declare_dram_parameter("qkx", [P, NJ, 2, EO, NCH], f8, isOutput=False)
    vx_e = nc.declare_dram_parameter("vx", [P, NJ, EO, NCH], f16, isOutput=False)
    wqk_e = nc.declare_dram_parameter("wqk", [P, 2, EO, DQ], f8, isOutput=False)
    wv_e = nc.declare_dram_parameter("wv", [P, EO, DV], f16, isOutput=False)
    bias3_e = nc.declare_dram_parameter("bias3", [P, 3], f32, isOutput=False)
    idmask_e = nc.declare_dram_parameter("idmask", [P, 2, P], f16, isOutput=False)
    out_e = nc.declare_dram_parameter("out", [P, ST, DV], f16, isOutput=True)

    Exp = mybir.ActivationFunctionType.Exp

    with (
        tile.TileContext(nc) as tc,
        tc.tile_pool(name="consts", bufs=1) as consts,
        tc.tile_pool(name="inx", bufs=9) as inx,
        tc.tile_pool(name="acts", bufs=1) as acts,
        tc.tile_pool(name="outp", bufs=10) as outp,
        tc.tile_pool(name="pp", bufs=3, space="PSUM") as pp,
        tc.tile_pool(name="ps_s", bufs=3, space="PSUM") as ps_s_pool,
        tc.tile_pool(name="ps_n", bufs=2, space="PSUM") as ps_n_pool,
    ):
        # Consts ride the scalar HWDGE / gpsimd SWDGE so the sync HWDGE ring
        # carries nothing but streamed input chunks (no head-of-line waits).
        # wq leads: it gates the very first projection matmul; ident/mask are
        # not needed until mid-round-0 so they trail the weights.
        wqk_sb = consts.tile([P, 2, EO, DQ], f8, tag="wqk")
        nc.scalar.dma_start(wqk_sb[:], wqk_e.ap())
        wv_sb = consts.tile([P, EO, DQ], f16, tag="wv")
        nc.scalar.dma_start(wv_sb[:], wv_e.ap())
        bias3 = consts.tile([P, 3], f32, tag="bias3")
        nc.gpsimd.dma_start(bias3[:], bias3_e.ap())
        idmask = consts.tile([P, 2, P], f16, tag="idmask")
        nc.scalar.dma_start(idmask[:], idmask_e.ap())
        id_sb = idmask[:, 0, :]
        mask_sb = idmask[:, 1, :]
        b_sb = {
            "bq": bias3[:, 0:1],
            "bk": bias3[:, 1:2],
            "bv": bias3[:, 2:3],
        }

        # PE warm-up: the first real matmul cannot start before the first
        # input chunk lands (~3.5us), and a cold PE runs slow for its first
        # ~3us of activity.  A DMA-independent burst of matmuls on a memset
        # operand spans the wait so the HAM clock gate is warm when the
        # projections begin.  One junk DVE copy consumes the result so DCE
        # keeps it.
        wu_in = consts.tile([P, NCH], f16, tag="wu_in")
        nc.vector.memset(wu_in[:], 1.0)
        wu_ps = pp.tile([P, NCH], f32, tag="pp")
        junk = consts.tile([P, P], f32, tag="junk")
        for _ in range(12):
            nc.tensor.matmul(wu_ps[:], wu_in[:, :P], wu_in[:], start=True, stop=True)

        qT = acts.tile([P, S], f16, tag="qT")
        kT = acts.tile([P, S], f16, tag="kT")
        vT = acts.tile([P, S], f16, tag="vT")
        v_ext = acts.tile([P, ST, DV + 1], f16, tag="vex")
        nc.vector.memset(v_ext[:, :, DV : DV + 1], 1.0)
        E_big = acts.tile([P, ST, S], f16, tag="exp")
        obuf = acts.tile([P, ST, DV], f16, tag="obuf")

        # ring plan (per 128-desc transfer the ring is busy ~1.3us, so rings
        # are the scarce resource early):
        #   sync:   packed qk chunks (one transfer carries q_j AND k_j), stores
        #   scalar: wqk, wv, then v chunks
        #   gpsimd: packed bias3 + packed ident/mask
        def proj_qk(j):
            xc = inx.tile([P, 2, EO, NCH], f8, tag="qk")
            nc.sync.dma_start(xc[:], qkx_e.ap()[:, j])
            sl = slice(j * NCH, (j + 1) * NCH)
            for half, bt, dst in ((0, b_sb["bq"], qT), (1, b_sb["bk"], kT)):
                ps = pp.tile([P, NCH], f32, tag="pp")
                # fp8 DoubleRow: two 128-deep e-tiles per pass
                for h in range(EO // 2):
                    nc.tensor.matmul(
                        ps[:],
                        wqk_sb[:, half, 2 * h : 2 * h + 2, :],
                        xc[:, half, 2 * h : 2 * h + 2, :],
                        start=(h == 0),
                        stop=(h == EO // 2 - 1),
                        perf_mode=DR,
                    )
                nc.vector.tensor_scalar_add(dst[:, sl], ps[:], bt)

        vtiles = {}

        def v_dma(j):
            # DMA triggers run on their engine's instruction queue, so a ring
            # shared with data-dependent compute is paced by that compute.
            # Even v chunks ride the otherwise-idle gpsimd SWDGE (its queue
            # races ahead and fires triggers immediately); odd ones ride the
            # scalar ring, issued a round early so they sit in front of that
            # round's exp instructions.
            xc = inx.tile([P, EO, NCH], f16, tag="inx2")
            eng = nc.gpsimd if j % 2 == 0 else nc.scalar
            eng.dma_start(xc[:], vx_e.ap()[:, j])
            vtiles[j] = xc

        def proj_v(j):
            xc = vtiles.pop(j)
            ps = pp.tile([P, NCH], f32, tag="pp")
            sl = slice(j * NCH, (j + 1) * NCH)
            for eo in range(EO):
                nc.tensor.matmul(
                    ps[:],
                    wv_sb[:, eo, :],
                    xc[:, eo, :],
                    start=(eo == 0),
                    stop=(eo == EO - 1),
                )
            nc.vector.tensor_scalar_add(vT[:, sl], ps[:], b_sb["bv"])

        def v_round(r):
            # v chunk + projection + [s, dv] v blocks + diagonal masks for
            # round r (the v DMA stays in its own round so the stream order
            # is unchanged)
            proj_v(r)
            for st in range(TPR * r, TPR * (r + 1)):
                tp = ps_n_pool.tile([P, P], f16, tag="ps_n")
                nc.tensor.transpose(tp[:], vT[:, st * P : (st + 1) * P], id_sb)
                nc.vector.tensor_copy(v_ext[:, st, 0:DV], tp[:])
            for kt in range(TPR * r, TPR * (r + 1)):
                d0 = kt * P
                nc.vector.tensor_mul(
                    E_big[:, kt, d0 : d0 + P],
                    E_big[:, kt, d0 : d0 + P],
                    mask_sb,
                )

        def av_round(r):
            # AV + normalize for round r's q tiles; runs one round behind its
            # scores (at the TOP of round r+1) so the ScalarE exp stream is
            # never on the AV critical path and the in-order PE has ready
            # work to chew while round r+1's chunks are still streaming in
            for qt in range(TPR * r, TPR * (r + 1)):
                pn = ps_n_pool.tile([P, DV + 1], f32, tag="ps_n")
                for kt in range(qt + 1):
                    nc.tensor.matmul(
                        pn[:],
                        E_big[:, kt, qt * P : (qt + 1) * P],
                        v_ext[:, kt, :],
                        start=(kt == 0),
                        stop=(kt == qt),
                    )
                rec = outp.tile([P, 1], f32, tag="rec")
                nc.vector.reciprocal(rec[:], pn[:, DV : DV + 1])
                nc.vector.tensor_scalar_mul(obuf[:, qt, :], pn[:, 0:DV], rec[:])

        for j in range(NJ):
            with nc.named_scope(f"round{j}"):
                if j == 0:
                    v_dma(0)
                if j < NJ - 1:
                    v_dma(j + 1)
                if j >= 1:
                    av_round(j - 1)
                # qk projections first: scores depend on them, and their
                # packed chunk gets the early DMA bandwidth; v follows.
                proj_qk(j)

                # rounds 0-1 are DMA-paced: filler matmuls keep the PE busy
                # enough that the HAM activity monitor never re-throttles the
                # clock while the input stream catches up
                if j <= 1:
                    for _ in range(6 if j == 0 else 4):
                        nc.tensor.matmul(
                            wu_ps[:], wu_in[:, :P], wu_in[:], start=True, stop=True
                        )
                    if j == 1:
                        nc.vector.tensor_copy(junk[:], wu_ps[:, :P])

                # scores^T for q-chunk j against all causal k tiles; two
                # k-tiles share one PSUM pair-tile so a single exp call covers
                # both (amortizes ACT per-instruction overhead).
                sl = slice(j * NCH, (j + 1) * NCH)
                for kt in range(0, TPR * (j + 1), 2):
                    ps = ps_s_pool.tile([P, 2, NCH], f32, tag="ps_s")
                    for u in range(2):
                        nc.tensor.matmul(
                            ps[:, u, :],
                            kT[:, (kt + u) * P : (kt + u + 1) * P],
                            qT[:, sl],
                            start=True,
                            stop=True,
                        )
                    nc.scalar.activation(
                        E_big[:, kt : kt + 2, sl], ps[:], Exp, scale=SCALE
                    )

                v_round(j)
                if j == NJ - 1:
                    # tiles 0-13 are normalized by now; the sync ring is idle
                    nc.sync.dma_start(out_e.ap()[:, : ST - 2], obuf[:, : ST - 2])

        with nc.named_scope("avtail"):
            av_round(NJ - 1)
            nc.sync.dma_start(out_e.ap()[:, ST - 2 :], obuf[:, ST - 2 :])

    nc.compile()
    return nc


def _get_nc():
    if "nc" not in _CACHE:
        _CACHE["nc"] = _build_nc()
    return _CACHE["nc"]


NP_F8 = mybir.dt.np(f8)


def _prep_consts(Wq, bq, Wk, bk, Wv, bv):
    def prep_w(W, scale, npdt):  # [D, E] f32 -> (scale*W).T [E, D] -> [ei, eo, D]
        WT = (scale * W).T.astype(npdt)  # [E, D]
        return np.ascontiguousarray(WT.reshape(EO, P, -1).transpose(1, 0, 2))

    consts = {
        "wqk": np.ascontiguousarray(
            np.stack([prep_w(Wq, WS, NP_F8), prep_w(Wk, WS, NP_F8)], axis=1)
        ),
        "wv": prep_w(Wv, 1.0, np.float16),
        "bias3": np.ascontiguousarray(
            np.stack([WS * bq, WS * bk, bv], axis=1).astype(np.float32)
        ),
        "idmask": np.ascontiguousarray(
            np.stack(
                [np.eye(P, dtype=np.float16), np.triu(np.ones((P, P), np.float16))],
                axis=1,
            )
        ),
    }
    return consts


def _prep_x(x, npdt):  # [S, E] f32 -> xT [E, S] -> [ei, j, eo, s_in_chunk]
    xT = x.astype(npdt).T  # [E, S]
    x4 = xT.reshape(EO, P, NJ, NCH)  # [eo, ei, j, s]
    return np.ascontiguousarray(x4.transpose(1, 2, 0, 3))


def kernel(query, key_in, value, Wq, bq, Wk, bk, Wv, bv):
    global LAST_RESULT
    query = np.asarray(query, dtype=np.float32)
    key_in = np.asarray(key_in, dtype=np.float32)
    value = np.asarray(value, dtype=np.float32)
    consts = _prep_consts(
        np.asarray(Wq), np.asarray(bq), np.asarray(Wk),
        np.asarray(bk), np.asarray(Wv), np.asarray(bv),
    )
    in_maps = []
    for b in range(NCORES):
        m = dict(consts)
        m["qkx"] = np.ascontiguousarray(
            np.stack(
                [_prep_x(query[b], NP_F8), _prep_x(key_in[b], NP_F8)], axis=2
            )
        )
        m["vx"] = _prep_x(value[b], np.float16)
        in_maps.append(m)

    nc = _get_nc()
    res = run_bass_kernel_spmd(nc, in_maps, core_ids=list(range(NCORES)))
    LAST_RESULT = res
    outs = []
    for i in range(NCORES):
        o = res.results[i]["out"]  # [P, ST, DV] with s = st*P + ei
        outs.append(o.transpose(1, 0, 2).reshape(S, DV))
    return np.stack(outs, axis=0).astype(np.float32)


# revision 27
# speedup vs baseline: 1.1609x; 1.0994x over previous
"""Causal single-head attention (projections + softmax(QK^T)V) on 8 TRN2 cores.

Sharding: pure data parallelism over the batch dim (B=8 -> one batch element
per NeuronCore). Each core runs an identical Bass/Tile program on its shard.

Mixed-precision dataflow (chosen from a numpy error study against the fp32
reference; gate is rel_err < 2e-2):
  - query/key inputs stream as fp8 e4m3 [E,S] (halves their HBM traffic) and
    their projections run as fp8 DoubleRow matmuls (two 128-deep e-tiles
    contracted per pass -> 2x PE throughput).  Wq/Wk are pre-scaled by 32 on
    the host before e4m3 quantization so the uniform(-1/32,1/32) weights land
    in e4m3's normal range; the resulting 32*32=1024 score scale folds into
    the exp() scale constant for free.  q/k land in PSUM fp32 and evict to
    fp16, so the score matmul itself stays fp16 (no re-quantization).
  - the value path is the precision-critical one (v errors pass straight to
    the output for near-diagonal queries), so value streams fp16 and the
    v-projection + AV matmuls stay fp16 end to end.
  Measured (numpy sim, bit-faithful on the fp16 baseline): rel_err ~1.67e-2.

Per-core dataflow (everything transposed so no on-chip transposes of the big
score matrix are needed):
  - qT/kT/vT [d=128, S] = W_xT.T @ xT accumulated in PSUM fp32, bias added
    on the VectorE eviction to SBUF fp16,
  - scores^T block [k,q] = kT_blk.T @ qT, exp on ScalarE with fused
    1/(1024*sqrt(d)) scale (no max-subtraction: |scores_true/sqrt(d)| <= ~4
    so exp cannot overflow),
  - causal mask applied only on diagonal 128x128 blocks (entries with q < k
    in lower tiles are never read by the AV stage),
  - out[q, dv] and the softmax denominator come from one PE accumulation:
    [num | den] = exp_blk.T @ [v | 1]; normalize on VectorE; DMA out fp16,
    upcast to fp32 on the host.

The computation is pipelined in NJ rounds over 256-wide s-chunks: each round
loads its q/k/v input chunks (the sync HWDGE ring carries only these, in
order, so chunk DMAs complete just-in-time at full HBM bandwidth), projects
them, and immediately runs the newly-enabled score/exp/AV work.
"""

import math

import numpy as np

import concourse.bass as bass  # noqa: F401  (registers AP machinery)
import concourse.tile as tile
from concourse import bacc, mybir
from concourse.bass_utils import run_bass_kernel_spmd

B, S, E = 8, 2048, 1024
DQ, DV = 128, 128
P = 128
EO = E // P          # 8 e-chunks
ST = S // P          # 16 sequence tiles of 128
NCH = 256            # s-chunk width per pipeline round
NJ = S // NCH        # 8 s-chunks
TPR = ST // NJ       # sequence tiles per round (2)
NCORES = 8
WS = 32.0            # host-side Wq/Wk scale before fp8 quantization
SCALE = 1.0 / math.sqrt(DQ) / (WS * WS)

f8 = mybir.dt.float8e4
f16 = mybir.dt.float16
f32 = mybir.dt.float32
DR = mybir.MatmulPerfMode.DoubleRow

_CACHE = {}
LAST_RESULT = None  # BassKernelResults of the most recent run (for profiling)


def _build_nc():
    nc = bacc.Bacc("TRN2", target_bir_lowering=False, debug=False)

    qx_e = nc.declare_dram_parameter("qx", [P, NJ, EO, NCH], f8, isOutput=False)
    kx_e = nc.declare_dram_parameter("kx", [P, NJ, EO, NCH], f8, isOutput=False)
    vx_e = nc.declare_dram_parameter("vx", [P, NJ, EO, NCH], f16, isOutput=False)
    wq_e = nc.declare_dram_parameter("wq", [P, EO, DQ], f8, isOutput=False)
    wk_e = nc.declare_dram_parameter("wk", [P, EO, DQ], f8, isOutput=False)
    wv_e = nc.declare_dram_parameter("wv", [P, EO, DV], f16, isOutput=False)
    bias3_e = nc.declare_dram_parameter("bias3", [P, 3], f32, isOutput=False)
    idmask_e = nc.declare_dram_parameter("idmask", [P, 2, P], f16, isOutput=False)
    out_e = nc.declare_dram_parameter("out", [P, ST, DV], f16, isOutput=True)

    Exp = mybir.ActivationFunctionType.Exp

    with (
        tile.TileContext(nc) as tc,
        tc.tile_pool(name="consts", bufs=1) as consts,
        tc.tile_pool(name="inx", bufs=9) as inx,
        tc.tile_pool(name="acts", bufs=1) as acts,
        tc.tile_pool(name="outp", bufs=10) as outp,
        tc.tile_pool(name="pp", bufs=3, space="PSUM") as pp,
        tc.tile_pool(name="ps_s", bufs=3, space="PSUM") as ps_s_pool,
        tc.tile_pool(name="ps_n", bufs=2, space="PSUM") as ps_n_pool,
    ):
        # Consts ride the scalar HWDGE / gpsimd SWDGE so the sync HWDGE ring
        # carries nothing but streamed input chunks (no head-of-line waits).
        # wq leads: it gates the very first projection matmul; ident/mask are
        # not needed until mid-round-0 so they trail the weights.
        w_sb = {}
        for nm, ext, dt in (("wq", wq_e, f8), ("wk", wk_e, f8)):
            t = consts.tile([P, EO, DQ], dt, tag=nm)
            nc.scalar.dma_start(t[:], ext.ap())
            w_sb[nm] = t
        wv_sb = consts.tile([P, EO, DQ], f16, tag="wv")
        w_sb["wv"] = wv_sb
        bias3 = consts.tile([P, 3], f32, tag="bias3")
        nc.gpsimd.dma_start(bias3[:], bias3_e.ap())
        idmask = consts.tile([P, 2, P], f16, tag="idmask")
        nc.gpsimd.dma_start(idmask[:], idmask_e.ap())
        id_sb = idmask[:, 0, :]
        mask_sb = idmask[:, 1, :]
        b_sb = {
            "bq": bias3[:, 0:1],
            "bk": bias3[:, 1:2],
            "bv": bias3[:, 2:3],
        }

        # PE warm-up: the first real matmul cannot start before the first
        # input chunk lands (~3.5us), and a cold PE runs slow for its first
        # ~3us of activity.  A DMA-independent burst of matmuls on a memset
        # operand spans the wait so the HAM clock gate is warm when the
        # projections begin.  One junk DVE copy consumes the result so DCE
        # keeps it.
        wu_in = consts.tile([P, NCH], f16, tag="wu_in")
        nc.vector.memset(wu_in[:], 1.0)
        wu_ps = pp.tile([P, NCH], f32, tag="pp")
        junk = consts.tile([P, P], f32, tag="junk")
        for _ in range(12):
            nc.tensor.matmul(wu_ps[:], wu_in[:, :P], wu_in[:], start=True, stop=True)

        qT = acts.tile([P, S], f16, tag="qT")
        kT = acts.tile([P, S], f16, tag="kT")
        vT = acts.tile([P, S], f16, tag="vT")
        v_ext = acts.tile([P, ST, DV + 1], f16, tag="vex")
        nc.vector.memset(v_ext[:, :, DV : DV + 1], 1.0)
        E_big = acts.tile([P, ST, S], f16, tag="exp")
        obuf = acts.tile([P, ST, DV], f16, tag="obuf")

        proj_specs = (
            (qx_e, w_sb["wq"], b_sb["bq"], qT, f8),
            (kx_e, w_sb["wk"], b_sb["bk"], kT, f8),
            (vx_e, w_sb["wv"], b_sb["bv"], vT, f16),
        )

        def proj(j, ti):
            xe, wt, bt, dst, dt = proj_specs[ti]
            xc = inx.tile([P, EO, NCH], dt, tag=f"inx{ti}")
            # ring plan (per 128-desc transfer the ring is busy ~1.3us, so
            # rings are the scarce resource early):
            #   sync:   q/v chunks (+ wv between q0 and q1), stores
            #   scalar: wq, wk, then k chunks
            #   gpsimd: packed bias3 + packed ident/mask
            xeng = nc.scalar if ti == 1 else nc.sync
            xeng.dma_start(xc[:], xe.ap()[:, j])
            ps = pp.tile([P, NCH], f32, tag="pp")
            sl = slice(j * NCH, (j + 1) * NCH)
            if dt is f8:
                # fp8 DoubleRow: two 128-deep e-tiles per pass
                for h in range(EO // 2):
                    nc.tensor.matmul(
                        ps[:],
                        wt[:, 2 * h : 2 * h + 2, :],
                        xc[:, 2 * h : 2 * h + 2, :],
                        start=(h == 0),
                        stop=(h == EO // 2 - 1),
                        perf_mode=DR,
                    )
            else:
                for eo in range(EO):
                    nc.tensor.matmul(
                        ps[:],
                        wt[:, eo, :],
                        xc[:, eo, :],
                        start=(eo == 0),
                        stop=(eo == EO - 1),
                    )
            nc.vector.tensor_scalar_add(dst[:, sl], ps[:], bt)

        def v_round(r):
            # v chunk + projection + [s, dv] v blocks + diagonal masks for
            # round r (the v DMA stays in its own round so the stream order
            # is unchanged)
            proj(r, 2)
            for st in range(TPR * r, TPR * (r + 1)):
                tp = ps_n_pool.tile([P, P], f16, tag="ps_n")
                nc.tensor.transpose(tp[:], vT[:, st * P : (st + 1) * P], id_sb)
                nc.vector.tensor_copy(v_ext[:, st, 0:DV], tp[:])
            for kt in range(TPR * r, TPR * (r + 1)):
                d0 = kt * P
                nc.vector.tensor_mul(
                    E_big[:, kt, d0 : d0 + P],
                    E_big[:, kt, d0 : d0 + P],
                    mask_sb,
                )

        def av_round(r):
            # AV + normalize for round r's q tiles; runs one round behind its
            # scores (at the TOP of round r+1) so the ScalarE exp stream is
            # never on the AV critical path and the in-order PE has ready
            # work to chew while round r+1's chunks are still streaming in
            for qt in range(TPR * r, TPR * (r + 1)):
                pn = ps_n_pool.tile([P, DV + 1], f32, tag="ps_n")
                for kt in range(qt + 1):
                    nc.tensor.matmul(
                        pn[:],
                        E_big[:, kt, qt * P : (qt + 1) * P],
                        v_ext[:, kt, :],
                        start=(kt == 0),
                        stop=(kt == qt),
                    )
                rec = outp.tile([P, 1], f32, tag="rec")
                nc.vector.reciprocal(rec[:], pn[:, DV : DV + 1])
                nc.vector.tensor_scalar_mul(obuf[:, qt, :], pn[:, 0:DV], rec[:])

        for j in range(NJ):
            with nc.named_scope(f"round{j}"):
                if j >= 1:
                    av_round(j - 1)
                # q and k projections first: scores depend on them, and their
                # chunks get the early DMA bandwidth; v follows the scores.
                proj(j, 0)
                proj(j, 1)
                if j == 0:
                    # wv rides the sync ring behind q0
                    nc.sync.dma_start(w_sb["wv"][:], wv_e.ap())

                # rounds 0-1 are DMA-paced: filler matmuls keep the PE busy
                # enough that the HAM activity monitor never re-throttles the
                # clock while the input stream catches up
                if j <= 1:
                    for _ in range(6 if j == 0 else 4):
                        nc.tensor.matmul(
                            wu_ps[:], wu_in[:, :P], wu_in[:], start=True, stop=True
                        )
                    if j == 1:
                        nc.vector.tensor_copy(junk[:], wu_ps[:, :P])

                # scores^T for q-chunk j against all causal k tiles; two
                # k-tiles share one PSUM pair-tile so a single exp call covers
                # both (amortizes ACT per-instruction overhead).
                sl = slice(j * NCH, (j + 1) * NCH)
                for kt in range(0, TPR * (j + 1), 2):
                    ps = ps_s_pool.tile([P, 2, NCH], f32, tag="ps_s")
                    for u in range(2):
                        nc.tensor.matmul(
                            ps[:, u, :],
                            kT[:, (kt + u) * P : (kt + u + 1) * P],
                            qT[:, sl],
                            start=True,
                            stop=True,
                        )
                    nc.scalar.activation(
                        E_big[:, kt : kt + 2, sl], ps[:], Exp, scale=SCALE
                    )

                v_round(j)
                if j == NJ - 1:
                    # tiles 0-13 are normalized by now; the sync ring is idle
                    nc.sync.dma_start(out_e.ap()[:, : ST - 2], obuf[:, : ST - 2])

        with nc.named_scope("avtail"):
            av_round(NJ - 1)
            nc.sync.dma_start(out_e.ap()[:, ST - 2 :], obuf[:, ST - 2 :])

    nc.compile()
    return nc


def _get_nc():
    if "nc" not in _CACHE:
        _CACHE["nc"] = _build_nc()
    return _CACHE["nc"]


NP_F8 = mybir.dt.np(f8)


def _prep_consts(Wq, bq, Wk, bk, Wv, bv):
    def prep_w(W, scale, npdt):  # [D, E] f32 -> (scale*W).T [E, D] -> [ei, eo, D]
        WT = (scale * W).T.astype(npdt)  # [E, D]
        return np.ascontiguousarray(WT.reshape(EO, P, -1).transpose(1, 0, 2))

    consts = {
        "wq": prep_w(Wq, WS, NP_F8),
        "wk": prep_w(Wk, WS, NP_F8),
        "wv": prep_w(Wv, 1.0, np.float16),
        "bias3": np.ascontiguousarray(
            np.stack([WS * bq, WS * bk, bv], axis=1).astype(np.float32)
        ),
        "idmask": np.ascontiguousarray(
            np.stack(
                [np.eye(P, dtype=np.float16), np.triu(np.ones((P, P), np.float16))],
                axis=1,
            )
        ),
    }
    return consts


def _prep_x(x, npdt):  # [S, E] f32 -> xT [E, S] -> [ei, j, eo, s_in_chunk]
    xT = x.astype(npdt).T  # [E, S]
    x4 = xT.reshape(EO, P, NJ, NCH)  # [eo, ei, j, s]
    return np.ascontiguousarray(x4.transpose(1, 2, 0, 3))


def kernel(query, key_in, value, Wq, bq, Wk, bk, Wv, bv):
    global LAST_RESULT
    query = np.asarray(query, dtype=np.float32)
    key_in = np.asarray(key_in, dtype=np.float32)
    value = np.asarray(value, dtype=np.float32)
    consts = _prep_consts(
        np.asarray(Wq), np.asarray(bq), np.asarray(Wk),
        np.asarray(bk), np.asarray(Wv), np.asarray(bv),
    )
    in_maps = []
    for b in range(NCORES):
        m = dict(consts)
        m["qx"] = _prep_x(query[b], NP_F8)
        m["kx"] = _prep_x(key_in[b], NP_F8)
        m["vx"] = _prep_x(value[b], np.float16)
        in_maps.append(m)

    nc = _get_nc()
    res = run_bass_kernel_spmd(nc, in_maps, core_ids=list(range(NCORES)))
    LAST_RESULT = res
    outs = []
    for i in range(NCORES):
        o = res.results[i]["out"]  # [P, ST, DV] with s = st*P + ei
        outs.append(o.transpose(1, 0, 2).reshape(S, DV))
    return np.stack(outs, axis=0).astype(np.float32)


# revision 28
# speedup vs baseline: 1.1744x; 1.0116x over previous
"""Causal single-head attention (projections + softmax(QK^T)V) on 8 TRN2 cores.

Sharding: pure data parallelism over the batch dim (B=8 -> one batch element
per NeuronCore). Each core runs an identical Bass/Tile program on its shard.

Mixed-precision dataflow (chosen from a numpy error study against the fp32
reference; gate is rel_err < 2e-2):
  - query/key inputs stream as fp8 e4m3 [E,S] (halves their HBM traffic) and
    their projections run as fp8 DoubleRow matmuls (two 128-deep e-tiles
    contracted per pass -> 2x PE throughput).  Wq/Wk are pre-scaled by 32 on
    the host before e4m3 quantization so the uniform(-1/32,1/32) weights land
    in e4m3's normal range; the resulting 32*32=1024 score scale folds into
    the exp() scale constant for free.  q/k land in PSUM fp32 and evict to
    fp16, so the score matmul itself stays fp16 (no re-quantization).
  - the value path is the precision-critical one (v errors pass straight to
    the output for near-diagonal queries), so value streams fp16 and the
    v-projection + AV matmuls stay fp16 end to end.
  Measured (numpy sim, bit-faithful on the fp16 baseline): rel_err ~1.67e-2.

Per-core dataflow (everything transposed so no on-chip transposes of the big
score matrix are needed):
  - qT/kT/vT [d=128, S] = W_xT.T @ xT accumulated in PSUM fp32, bias added
    on the VectorE eviction to SBUF fp16,
  - scores^T block [k,q] = kT_blk.T @ qT, exp on ScalarE with fused
    1/(1024*sqrt(d)) scale (no max-subtraction: |scores_true/sqrt(d)| <= ~4
    so exp cannot overflow),
  - causal mask applied only on diagonal 128x128 blocks (entries with q < k
    in lower tiles are never read by the AV stage),
  - out[q, dv] and the softmax denominator come from one PE accumulation:
    [num | den] = exp_blk.T @ [v | 1]; normalize on VectorE; DMA out fp16,
    upcast to fp32 on the host.

The computation is pipelined in NJ rounds over 256-wide s-chunks: each round
loads its q/k/v input chunks (the sync HWDGE ring carries only these, in
order, so chunk DMAs complete just-in-time at full HBM bandwidth), projects
them, and immediately runs the newly-enabled score/exp/AV work.
"""

import math

import numpy as np

import concourse.bass as bass  # noqa: F401  (registers AP machinery)
import concourse.tile as tile
from concourse import bacc, mybir
from concourse.bass_utils import run_bass_kernel_spmd

B, S, E = 8, 2048, 1024
DQ, DV = 128, 128
P = 128
EO = E // P          # 8 e-chunks
ST = S // P          # 16 sequence tiles of 128
NCH = 256            # s-chunk width per pipeline round
NJ = S // NCH        # 8 s-chunks
TPR = ST // NJ       # sequence tiles per round (2)
NCORES = 8
WS = 32.0            # host-side Wq/Wk scale before fp8 quantization
SCALE = 1.0 / math.sqrt(DQ) / (WS * WS)

f8 = mybir.dt.float8e4
f16 = mybir.dt.float16
f32 = mybir.dt.float32
DR = mybir.MatmulPerfMode.DoubleRow

_CACHE = {}
LAST_RESULT = None  # BassKernelResults of the most recent run (for profiling)


def _build_nc():
    nc = bacc.Bacc("TRN2", target_bir_lowering=False, debug=False)

    qx_e = nc.declare_dram_parameter("qx", [P, NJ, EO, NCH], f8, isOutput=False)
    kx_e = nc.declare_dram_parameter("kx", [P, NJ, EO, NCH], f8, isOutput=False)
    vx_e = nc.declare_dram_parameter("vx", [P, NJ, EO, NCH], f16, isOutput=False)
    wq_e = nc.declare_dram_parameter("wq", [P, EO, DQ], f8, isOutput=False)
    wk_e = nc.declare_dram_parameter("wk", [P, EO, DQ], f8, isOutput=False)
    wv_e = nc.declare_dram_parameter("wv", [P, EO, DV], f16, isOutput=False)
    bias3_e = nc.declare_dram_parameter("bias3", [P, 3], f32, isOutput=False)
    idmask_e = nc.declare_dram_parameter("idmask", [P, 2, P], f16, isOutput=False)
    out_e = nc.declare_dram_parameter("out", [P, ST, DV], f16, isOutput=True)

    Exp = mybir.ActivationFunctionType.Exp

    with (
        tile.TileContext(nc) as tc,
        tc.tile_pool(name="sb", bufs=1) as sb_pool,
        tc.tile_pool(name="psum", bufs=1, space="PSUM") as psum_pool,
    ):
        consts = sb_pool
        acts = sb_pool
        inx = sb_pool
        outp = sb_pool
        pp = psum_pool
        ps_s_pool = psum_pool
        ps_n_pool = psum_pool
        # Consts ride the scalar HWDGE / gpsimd SWDGE so the sync HWDGE ring
        # carries nothing but streamed input chunks (no head-of-line waits).
        # wq leads: it gates the very first projection matmul; ident/mask are
        # not needed until mid-round-0 so they trail the weights.
        w_sb = {}
        for nm, ext, dt in (("wq", wq_e, f8), ("wk", wk_e, f8)):
            t = consts.tile([P, EO, DQ], dt, tag=nm)
            nc.scalar.dma_start(t[:], ext.ap())
            w_sb[nm] = t
        wv_sb = consts.tile([P, EO, DQ], f16, tag="wv")
        w_sb["wv"] = wv_sb
        bias3 = consts.tile([P, 3], f32, tag="bias3")
        nc.gpsimd.dma_start(bias3[:], bias3_e.ap())
        idmask = consts.tile([P, 2, P], f16, tag="idmask")
        nc.gpsimd.dma_start(idmask[:], idmask_e.ap())
        id_sb = idmask[:, 0, :]
        mask_sb = idmask[:, 1, :]
        b_sb = {
            "bq": bias3[:, 0:1],
            "bk": bias3[:, 1:2],
            "bv": bias3[:, 2:3],
        }

        # PE warm-up: the first real matmul cannot start before the first
        # input chunk lands (~3.5us), and a cold PE runs slow for its first
        # ~3us of activity.  A DMA-independent burst of matmuls on a memset
        # operand spans the wait so the HAM clock gate is warm when the
        # projections begin.  One junk DVE copy consumes the result so DCE
        # keeps it.
        wu_in = consts.tile([P, NCH], f16, tag="wu_in")
        nc.vector.memset(wu_in[:], 1.0)
        wu_ps = pp.tile([P, NCH], f32, tag="pp", bufs=3)
        junk = consts.tile([P, P], f32, tag="junk")
        for _ in range(12):
            nc.tensor.matmul(wu_ps[:], wu_in[:, :P], wu_in[:], start=True, stop=True)

        qT = acts.tile([P, S], f16, tag="qT")
        kT = acts.tile([P, S], f16, tag="kT")
        vT = acts.tile([P, S], f16, tag="vT")
        v_ext = acts.tile([P, ST, DV + 1], f16, tag="vex")
        nc.vector.memset(v_ext[:, :, DV : DV + 1], 1.0)
        E_big = acts.tile([P, ST, S], f16, tag="exp")
        obuf = acts.tile([P, ST, DV], f16, tag="obuf")

        proj_specs = (
            (qx_e, w_sb["wq"], b_sb["bq"], qT, f8),
            (kx_e, w_sb["wk"], b_sb["bk"], kT, f8),
            (vx_e, w_sb["wv"], b_sb["bv"], vT, f16),
        )

        def proj(j, ti):
            xe, wt, bt, dst, dt = proj_specs[ti]
            xc = inx.tile([P, EO, NCH], dt, tag=f"inx{ti}", bufs=9)
            # ring plan (per 128-desc transfer the ring is busy ~1.3us, so
            # rings are the scarce resource early):
            #   sync:   q/v chunks (+ wv between q0 and q1), stores
            #   scalar: wq, wk, then k chunks
            #   gpsimd: packed bias3 + packed ident/mask
            xeng = nc.scalar if ti == 1 else nc.sync
            xeng.dma_start(xc[:], xe.ap()[:, j])
            ps = pp.tile([P, NCH], f32, tag="pp", bufs=3)
            sl = slice(j * NCH, (j + 1) * NCH)
            if dt is f8:
                # fp8 DoubleRow: two 128-deep e-tiles per pass
                for h in range(EO // 2):
                    nc.tensor.matmul(
                        ps[:],
                        wt[:, 2 * h : 2 * h + 2, :],
                        xc[:, 2 * h : 2 * h + 2, :],
                        start=(h == 0),
                        stop=(h == EO // 2 - 1),
                        perf_mode=DR,
                    )
            else:
                for eo in range(EO):
                    nc.tensor.matmul(
                        ps[:],
                        wt[:, eo, :],
                        xc[:, eo, :],
                        start=(eo == 0),
                        stop=(eo == EO - 1),
                    )
            nc.vector.tensor_scalar_add(dst[:, sl], ps[:], bt)

        def v_round(r):
            # v chunk + projection + [s, dv] v blocks + diagonal masks for
            # round r (the v DMA stays in its own round so the stream order
            # is unchanged)
            proj(r, 2)
            for st in range(TPR * r, TPR * (r + 1)):
                tp = ps_n_pool.tile([P, P], f16, tag="ps_n", bufs=2)
                nc.tensor.transpose(tp[:], vT[:, st * P : (st + 1) * P], id_sb)
                nc.vector.tensor_copy(v_ext[:, st, 0:DV], tp[:])
            for kt in range(TPR * r, TPR * (r + 1)):
                d0 = kt * P
                nc.vector.tensor_mul(
                    E_big[:, kt, d0 : d0 + P],
                    E_big[:, kt, d0 : d0 + P],
                    mask_sb,
                )

        def av_round(r):
            # AV + normalize for round r's q tiles; runs one round behind its
            # scores (at the TOP of round r+1) so the ScalarE exp stream is
            # never on the AV critical path and the in-order PE has ready
            # work to chew while round r+1's chunks are still streaming in
            for qt in range(TPR * r, TPR * (r + 1)):
                pn = ps_n_pool.tile([P, DV + 1], f32, tag="ps_n", bufs=2)
                for kt in range(qt + 1):
                    nc.tensor.matmul(
                        pn[:],
                        E_big[:, kt, qt * P : (qt + 1) * P],
                        v_ext[:, kt, :],
                        start=(kt == 0),
                        stop=(kt == qt),
                    )
                rec = outp.tile([P, 1], f32, tag="rec", bufs=10)
                nc.vector.reciprocal(rec[:], pn[:, DV : DV + 1])
                nc.vector.tensor_scalar_mul(obuf[:, qt, :], pn[:, 0:DV], rec[:])

        for j in range(NJ):
            with nc.named_scope(f"round{j}"):
                if j >= 1:
                    av_round(j - 1)
                # q and k projections first: scores depend on them, and their
                # chunks get the early DMA bandwidth; v follows the scores.
                proj(j, 0)
                proj(j, 1)
                if j == 0:
                    # wv rides the sync ring behind q0
                    nc.sync.dma_start(w_sb["wv"][:], wv_e.ap())

                # rounds 0-1 are DMA-paced: filler matmuls keep the PE busy
                # enough that the HAM activity monitor never re-throttles the
                # clock while the input stream catches up
                if j <= 1:
                    for _ in range(6 if j == 0 else 4):
                        nc.tensor.matmul(
                            wu_ps[:], wu_in[:, :P], wu_in[:], start=True, stop=True
                        )
                    if j == 1:
                        nc.vector.tensor_copy(junk[:], wu_ps[:, :P])

                # scores^T for q-chunk j against all causal k tiles; two
                # k-tiles share one PSUM pair-tile so a single exp call covers
                # both (amortizes ACT per-instruction overhead).
                sl = slice(j * NCH, (j + 1) * NCH)
                for kt in range(0, TPR * (j + 1), 2):
                    ps = ps_s_pool.tile([P, 2, NCH], f32, tag="ps_s", bufs=3)
                    for u in range(2):
                        nc.tensor.matmul(
                            ps[:, u, :],
                            kT[:, (kt + u) * P : (kt + u + 1) * P],
                            qT[:, sl],
                            start=True,
                            stop=True,
                        )
                    nc.scalar.activation(
                        E_big[:, kt : kt + 2, sl], ps[:], Exp, scale=SCALE
                    )

                v_round(j)
                if j == NJ - 1:
                    # tiles 0-13 are normalized by now; the sync ring is idle
                    nc.sync.dma_start(out_e.ap()[:, : ST - 2], obuf[:, : ST - 2])

        with nc.named_scope("avtail"):
            av_round(NJ - 1)
            nc.sync.dma_start(out_e.ap()[:, ST - 2 :], obuf[:, ST - 2 :])

    nc.compile()
    return nc


def _get_nc():
    if "nc" not in _CACHE:
        _CACHE["nc"] = _build_nc()
    return _CACHE["nc"]


NP_F8 = mybir.dt.np(f8)


def _prep_consts(Wq, bq, Wk, bk, Wv, bv):
    def prep_w(W, scale, npdt):  # [D, E] f32 -> (scale*W).T [E, D] -> [ei, eo, D]
        WT = (scale * W).T.astype(npdt)  # [E, D]
        return np.ascontiguousarray(WT.reshape(EO, P, -1).transpose(1, 0, 2))

    consts = {
        "wq": prep_w(Wq, WS, NP_F8),
        "wk": prep_w(Wk, WS, NP_F8),
        "wv": prep_w(Wv, 1.0, np.float16),
        "bias3": np.ascontiguousarray(
            np.stack([WS * bq, WS * bk, bv], axis=1).astype(np.float32)
        ),
        "idmask": np.ascontiguousarray(
            np.stack(
                [np.eye(P, dtype=np.float16), np.triu(np.ones((P, P), np.float16))],
                axis=1,
            )
        ),
    }
    return consts


def _prep_x(x, npdt):  # [S, E] f32 -> xT [E, S] -> [ei, j, eo, s_in_chunk]
    xT = x.astype(npdt).T  # [E, S]
    x4 = xT.reshape(EO, P, NJ, NCH)  # [eo, ei, j, s]
    return np.ascontiguousarray(x4.transpose(1, 2, 0, 3))


def kernel(query, key_in, value, Wq, bq, Wk, bk, Wv, bv):
    global LAST_RESULT
    query = np.asarray(query, dtype=np.float32)
    key_in = np.asarray(key_in, dtype=np.float32)
    value = np.asarray(value, dtype=np.float32)
    consts = _prep_consts(
        np.asarray(Wq), np.asarray(bq), np.asarray(Wk),
        np.asarray(bk), np.asarray(Wv), np.asarray(bv),
    )
    in_maps = []
    for b in range(NCORES):
        m = dict(consts)
        m["qx"] = _prep_x(query[b], NP_F8)
        m["kx"] = _prep_x(key_in[b], NP_F8)
        m["vx"] = _prep_x(value[b], np.float16)
        in_maps.append(m)

    nc = _get_nc()
    res = run_bass_kernel_spmd(nc, in_maps, core_ids=list(range(NCORES)))
    LAST_RESULT = res
    outs = []
    for i in range(NCORES):
        o = res.results[i]["out"]  # [P, ST, DV] with s = st*P + ei
        outs.append(o.transpose(1, 0, 2).reshape(S, DV))
    return np.stack(outs, axis=0).astype(np.float32)
